# revision 1
# baseline (speedup 1.0000x reference)
"""Trainium2 Bass kernel for nn_AttentionLayer. v3

Changes from v3 (v4):
  - den matmuls col-packed: den_a -> psum partition 0, den_b -> partition 32
    (tile_position col groups 0/1, concurrent on HW); summed in the epilogue.
  - reciprocal broadcast moved off the PE onto idle GPSIMD
    (partition_broadcast) - removes the rank-1 matmul + a beta-slot grab
    from the j-chunk boundary.

Changes from v2:
  - beta PSUM tiles are single-bank [128, 512] with bufs=4 and one exp per
    tile: gives the scheduler slack to issue score matmuls ahead of the
    previous pair's PV matmuls, so ACT exp overlaps PE instead of
    alternating with it.
  - prologue pipelined per 512-column chunk (DMA -> relu -> QKV overlap).
  - o_acc copied out of PSUM immediately at each j-chunk end.
"""

import numpy as np

import bass_rust
import concourse.bass as bass
import concourse.tile as tile
from concourse import mybir
from concourse.bass_utils import run_bass_kernel_spmd

N_CORES = 8
C = 256
M = 64
HW = 4096
JC = 512
N_JC = HW // JC
N_IT = HW // 128

F32 = mybir.dt.float32
F32R = mybir.dt.float32r
BF16 = mybir.dt.bfloat16


def _install_tile_drain_fix():
    def _drain_and_barrier(self, tick_clock, wait_clock):
        from concourse.tile import ScopedClock

        nc = self.nc
        probe = nc.sync.nop()
        wait_clock.add_sem_waits(
            probe.ins, ScopedClock({None: tick_clock.global_clock})
        )
        si = probe.ins.sync_info
        waits = list(si.on_wait) if si is not None else []
        probe.ins.sync_info = bass_rust.SyncInfo(on_wait=waits[:1], on_update=[])
        for w in waits[1:]:
            n = nc.sync.nop()
            n.ins.sync_info = bass_rust.SyncInfo(on_wait=[w], on_update=[])
        nc.sync.drain()
        nc.all_engine_barrier()
        assert self.sems is not None
        popped = nc._tile_sem_poison_stack.pop()
        assert popped is self._sem_poison
        nc.clear_and_free_semaphores(list(self.sems.allocated().values()))
        nc.all_engine_barrier()

    tile.TileContext._drain_and_barrier = _drain_and_barrier


def r(ap):
    return ap.bitcast(F32R)


def _split_multi_waits(nc):
    """walrus in this toolchain encodes at most one sync wait per
    instruction. Split any instruction carrying more onto single-wait
    NOPs inserted immediately before it on the same engine (in-order
    engines make this semantics-preserving)."""
    ctr = [0]

    def mk_nop(engine, wait):
        ctr[0] += 1
        n = mybir.InstNoOp(name=f"I-wsplit{ctr[0]}", ins=[], outs=[])
        n.engine = engine
        n.sync_info = bass_rust.SyncInfo(on_wait=[wait], on_update=[])
        return n

    for f in nc.m.functions:
        for bb in f.blocks:
            out = []
            changed = False
            for inst in bb.instructions:
                si = inst.sync_info
                waits = list(si.on_wait) if si is not None else []
                if len(waits) > 1:
                    for w in waits[:-1]:
                        out.append(mk_nop(inst.engine, w))
                    inst.sync_info = bass_rust.SyncInfo(
                        on_wait=[waits[-1]], on_update=list(si.on_update)
                    )
                    changed = True
                out.append(inst)
            if changed:
                bb.instructions = out



def build(split_waits=True):
    _install_tile_drain_fix()
    nc = bass.Bass("TRN2", target_bir_lowering=False, debug=False)

    x_ext = nc.declare_dram_parameter("x", [C, HW], F32, isOutput=False)
    w_ext = nc.declare_dram_parameter("Wqkv", [2 * M + C, C], F32, isOutput=False)
    b_ext = nc.declare_dram_parameter("bqkv", [2 * M + C, 1], F32, isOutput=False)
    g_ext = nc.declare_dram_parameter("gamma", [1, 1], F32, isOutput=False)
    out_ext = nc.declare_dram_parameter("out", [C, HW], F32, isOutput=True)

    ident_dram = nc.inline_tensor(np.eye(128, dtype=np.float32), "ident128")
    ones_dram = nc.inline_tensor(np.ones((128, 1), dtype=np.float32), "ones128")
    onesr_dram = nc.inline_tensor(np.ones((1, 128), dtype=np.float32), "onesrow")

    with tile.TileContext(nc) as tc:
        with (
            tc.tile_pool(name="const", bufs=1) as constp,
            tc.tile_pool(name="xin", bufs=1) as xp,
            tc.tile_pool(name="xr", bufs=1) as xrp,
            tc.tile_pool(name="wld", bufs=1) as wldp,
            tc.tile_pool(name="wt", bufs=1) as wtp,
            tc.tile_pool(name="qk", bufs=1) as qkp,
            tc.tile_pool(name="vt", bufs=1) as vtp,
            tc.tile_pool(name="e", bufs=8) as ep,
            tc.tile_pool(name="osb", bufs=3) as osbp,
            tc.tile_pool(name="misc", bufs=1) as miscp,
            tc.tile_pool(name="ps_b", bufs=5, space="PSUM") as psb,
            tc.tile_pool(name="ps_acc", bufs=1, space="PSUM") as psacc,
            tc.tile_pool(name="ps_misc", bufs=1, space="PSUM") as psmisc,
        ):
            # ---- constants ----
            ident = constp.tile([128, 128], F32)
            nc.sync.dma_start(ident[:], ident_dram.ap()[:, :])
            ones_col = constp.tile([128, 1], F32)
            nc.sync.dma_start(ones_col[:], ones_dram.ap()[:, :])
            ones_row = constp.tile([1, 128], F32)
            nc.sync.dma_start(ones_row[:], onesr_dram.ap()[:, :])
            ones_col_bf = constp.tile([128, 1], BF16)
            nc.vector.tensor_copy(ones_col_bf[:], ones_col[:])
            ones_row_bf = constp.tile([1, 128], BF16)
            nc.vector.tensor_copy(ones_row_bf[:], ones_row[:])

            # ---- load W rows, biases, gamma (before x so they win DMA) ----
            w_rows = []
            for oc in range(3):
                wt_ = wldp.tile([128, C], F32, tag=f"wrows{oc}", name=f"wrows{oc}")
                nc.sync.dma_start(wt_[:], w_ext.ap()[128 * oc : 128 * (oc + 1), :])
                w_rows.append(wt_)
            bias_qq = miscp.tile([128, 1], F32, tag="bqq")
            nc.sync.dma_start(bias_qq[0:64, :], b_ext.ap()[0:64, :])
            nc.sync.dma_start(bias_qq[64:128, :], b_ext.ap()[0:64, :])
            bias_kk = miscp.tile([128, 1], F32, tag="bkk")
            nc.sync.dma_start(bias_kk[0:64, :], b_ext.ap()[64:128, :])
            nc.sync.dma_start(bias_kk[64:128, :], b_ext.ap()[64:128, :])
            bias_v = []
            for cc in range(2):
                bv = miscp.tile([128, 1], F32, tag=f"bv{cc}", name=f"bv{cc}")
                nc.sync.dma_start(
                    bv[:], b_ext.ap()[128 + 128 * cc : 128 + 128 * (cc + 1), :]
                )
                bias_v.append(bv)
            gamma_sb = miscp.tile([1, 1], F32, tag="gam")
            nc.sync.dma_start(gamma_sb[:], g_ext.ap()[:, :])

            gamma_bf = miscp.tile([1, 1], BF16, tag="gambf")
            nc.vector.tensor_copy(gamma_bf[:], gamma_sb[:])
            g_ps = psmisc.tile([128, 1], F32, tag="psm")
            nc.tensor.matmul(g_ps[:], ones_row_bf[:], gamma_bf[:], start=True, stop=True)
            gamma_bc = miscp.tile([128, 1], F32, tag="gbc_sb")
            nc.vector.tensor_copy(gamma_bc[:], g_ps[:])
            gbv = []
            for cc in range(2):
                t = miscp.tile([128, 1], F32, tag=f"gbv{cc}", name=f"gbv{cc}")
                nc.vector.tensor_mul(t[:], bias_v[cc][:], gamma_bc[:])
                gbv.append(t)

            # ---- transpose W via PE ----
            wqqT = []
            wkkT = []
            wvT = []
            for cc in range(2):
                ps = psmisc.tile([128, 128], F32, tag="psm")
                nc.tensor.transpose(ps[:], w_rows[0][:, 128 * cc : 128 * (cc + 1)], ident[:])
                tq = wtp.tile([128, 128], BF16, tag=f"wqqT{cc}", name=f"wqqT{cc}")
                nc.vector.tensor_copy(tq[:, 0:64], ps[:, 0:64])
                nc.vector.tensor_copy(tq[:, 64:128], ps[:, 0:64])
                wqqT.append(tq)
                tk = wtp.tile([128, 128], BF16, tag=f"wkkT{cc}", name=f"wkkT{cc}")
                nc.vector.tensor_copy(tk[:, 0:64], ps[:, 64:128])
                nc.vector.tensor_copy(tk[:, 64:128], ps[:, 64:128])
                wkkT.append(tk)
            for cc in range(2):
                t = wtp.tile([128, 256], BF16, tag=f"wvT{cc}", name=f"wvT{cc}")
                for oc in range(2):
                    ps = psmisc.tile([128, 128], F32, tag="psm")
                    nc.tensor.transpose(
                        ps[:], w_rows[1 + oc][:, 128 * cc : 128 * (cc + 1)], ident[:]
                    )
                    nc.vector.tensor_copy(t[:, 128 * oc : 128 * (oc + 1)], ps[:])
                wvT.append(t)

            # ---- pipelined prologue + QKV per 512-col chunk ----
            x_sb = [
                xp.tile([128, HW], F32, tag=f"x{cc}", name=f"xchunk{cc}")
                for cc in range(2)
            ]
            xr_sb = [
                xrp.tile([128, HW], BF16, tag=f"xr{cc}", name=f"xrchunk{cc}")
                for cc in range(2)
            ]
            qq_sb = qkp.tile([128, HW], BF16, tag="qq")
            kk_sb = qkp.tile([128, HW], BF16, tag="kk")
            vtg = vtp.tile([128, N_IT * 256], BF16, tag="vtg")

            for n in range(N_JC):
                sl = slice(JC * n, JC * (n + 1))
                for cc in range(2):
                    nc.sync.dma_start(
                        x_sb[cc][:, sl], x_ext.ap()[128 * cc : 128 * (cc + 1), sl]
                    )
                    nc.scalar.activation(
                        xr_sb[cc][:, sl], x_sb[cc][:, sl],
                        mybir.ActivationFunctionType.Relu,
                    )
                ps = psb.tile([128, JC], F32, tag="beta", name="qqps")
                for kc in range(2):
                    nc.tensor.matmul(
                        ps[:], wqqT[kc][:], xr_sb[kc][:, sl],
                        start=(kc == 0), stop=(kc == 1),
                    )
                nc.vector.tensor_scalar_add(qq_sb[:, sl], ps[:], bias_qq[:])
                ps2 = psb.tile([128, JC], F32, tag="beta", name="kkps")
                for kc in range(2):
                    nc.tensor.matmul(
                        ps2[:], wkkT[kc][:], xr_sb[kc][:, sl],
                        start=(kc == 0), stop=(kc == 1),
                    )
                nc.vector.tensor_scalar_add(kk_sb[:, sl], ps2[:], bias_kk[:])
                # v^T for the 4 i-tiles of this chunk
                for tt in range(4 * n, 4 * (n + 1)):
                    psv = psb.tile([128, 256], F32, tag="beta", name="vtps")
                    for kc in range(2):
                        nc.tensor.matmul(
                            psv[:, 0:256],
                            xr_sb[kc][:, 128 * tt : 128 * (tt + 1)],
                            wvT[kc][:],
                            start=(kc == 0), stop=(kc == 1),
                        )
                    nc.vector.tensor_scalar_mul(
                        vtg[:, 256 * tt : 256 * (tt + 1)], psv[:, 0:256], gamma_bc[:]
                    )

            # ---- attention over j-chunks ----
            for jc in range(N_JC):
                jsl = slice(JC * jc, JC * (jc + 1))
                o_acc = [
                    psacc.tile([128, JC], F32, tag=f"oacc{cc}", name=f"oacc{cc}")
                    for cc in range(2)
                ]
                den = psmisc.tile([64, JC], F32, tag="psm", name="den")
                for tp in range(N_IT // 2):
                    it0, it1 = 2 * tp, 2 * tp + 1
                    # two K=64 score matmuls, concurrent via row tiling
                    ba = psb.tile([128, JC], F32, tag="beta", name="ba")
                    bb = psb.tile([128, JC], F32, tag="beta", name="bb")
                    nc.tensor.matmul(
                        ba[:],
                        qq_sb[0:64, 128 * it0 : 128 * (it0 + 1)],
                        kk_sb[0:64, jsl],
                        start=True, stop=True,
                    )
                    nc.tensor.matmul(
                        bb[:],
                        qq_sb[64:128, 128 * it1 : 128 * (it1 + 1)],
                        kk_sb[64:128, jsl],
                        start=True, stop=True,
                    )
                    for it, bt in ((it0, ba), (it1, bb)):
                        e_t = ep.tile([128, JC], BF16, tag="e", name="et")
                        nc.scalar.activation(
                            e_t[:], bt[:], mybir.ActivationFunctionType.Exp,
                            scale=0.125,
                        )
                        first = it == 0
                        last = it == N_IT - 1
                        for cc in range(2):
                            nc.tensor.matmul(
                                o_acc[cc][:],
                                vtg[:, 256 * it + 128 * cc : 256 * it + 128 * (cc + 1)],
                                e_t[:],
                                start=first, stop=last,
                            )
                        dsl = den[0:1, :] if it % 2 == 0 else den[32:33, :]
                        nc.tensor.matmul(
                            dsl,
                            ones_col_bf[:],
                            e_t[:],
                            start=(it < 2),
                            stop=(it >= N_IT - 2),
                        )

                # epilogue: free PSUM quickly, then normalize
                oc_sb = []
                for cc in range(2):
                    t = osbp.tile([128, JC], F32, tag=f"ocp{cc}", name=f"ocp{cc}")
                    nc.vector.tensor_copy(t[:], o_acc[cc][:])
                    oc_sb.append(t)
                dhalf = miscp.tile([1, JC], F32, tag="dhalf")
                nc.vector.tensor_copy(dhalf[:], den[0:1, :])
                dsum = miscp.tile([1, JC], F32, tag="dsum")
                nc.vector.tensor_add(dsum[:], dhalf[:], den[32:33, :])
                rden = miscp.tile([1, JC], F32, tag="rden")
                nc.vector.reciprocal(rden[:], dsum[:])
                rden_bf = miscp.tile([1, JC], BF16, tag="rdenbf")
                nc.vector.tensor_copy(rden_bf[:], rden[:])
                rb_ps = psb.tile([128, JC], F32, tag="beta", name="rbps")
                nc.tensor.matmul(rb_ps[:], ones_row_bf[:], rden_bf[:], start=True, stop=True)
                rb = osbp.tile([128, JC], F32, tag="rb", name="rb")
                nc.vector.tensor_copy(rb[:], rb_ps[:])
                for cc in range(2):
                    o_n = osbp.tile([128, JC], F32, tag="on")
                    nc.vector.tensor_mul(o_n[:], oc_sb[cc][:], rb[:])
                    res = osbp.tile([128, JC], F32, tag="res")
                    nc.vector.scalar_tensor_tensor(
                        res[:],
                        in0=o_n[:],
                        scalar=gbv[cc][:],
                        in1=x_sb[cc][:, jsl],
                        op0=mybir.AluOpType.add,
                        op1=mybir.AluOpType.add,
                    )
                    nc.sync.dma_start(
                        out_ext.ap()[128 * cc : 128 * (cc + 1), jsl], res[:]
                    )
    if split_waits:
        _split_multi_waits(nc)
    return nc


_NC_CACHE = None


def kernel(x, Wqkv, bqkv, gamma):
    global _NC_CACHE
    if _NC_CACHE is None:
        _NC_CACHE = build()
    nc = _NC_CACHE
    B = x.shape[0]
    assert B == N_CORES
    in_maps = []
    for i in range(B):
        in_maps.append(
            {
                "x": np.ascontiguousarray(x[i].reshape(C, HW), dtype=np.float32),
                "Wqkv": np.ascontiguousarray(Wqkv, dtype=np.float32),
                "bqkv": np.ascontiguousarray(np.asarray(bqkv).reshape(2 * M + C, 1), dtype=np.float32),
                "gamma": np.ascontiguousarray(np.asarray(gamma).reshape(1, 1), dtype=np.float32),
            }
        )
    res = run_bass_kernel_spmd(nc, in_maps, core_ids=list(range(N_CORES)))
    out = np.stack(
        [res.results[i]["out"].reshape(C, 64, 64) for i in range(N_CORES)]
    ).astype(np.float32)
    return out

